# revision 11
# baseline (speedup 1.0000x reference)
"""KNN flow interpolation (AccFlowSupervise) on 8 Trainium2 NeuronCores.

Problem: for each query point (pc0 transformed into pc1's ego frame), find the
3 nearest neighbors in pc1, inverse-distance-weight their flow vectors, and
also emit the ego-motion displacement (pose_flow).

Sharding: data-parallel over batch (B=2) x 4 query shards -> 8 cores. Each
core computes a [2048, 8192] negated-squared-distance matrix against the full
replicated reference cloud via one fused PE matmul (augmented K=5 inner dim:
-d2 = 2x.y - |x|^2 - |y|^2), takes per-row top-8 (InstMax) + indices
(InstMaxIndex) on the vector engine, gathers the 3 nearest flows with
indirect DMA, and combines with normalized inverse-distance weights.

The tiny pose math (4x4 inverse, [N,3] point transform, q2/r2 row norms and
pose_flow = pc0_t - pc0) is done host-side in fp32, matching the reference's
formulas; the O(N*M) work all runs on-device.
"""

import os
import sys
from contextlib import ExitStack

import numpy as np

for _p in ("/opt/trn_rl_repo", "/root/.axon_site/_ro/trn_rl_repo"):
    if os.path.isdir(_p) and _p not in sys.path:
        sys.path.append(_p)

import concourse.bass as bass
import concourse.tile as tile
from concourse import mybir
from concourse.bass_utils import run_bass_kernel_spmd

# Problem shape (hardcoded; see spec)
B, N, M = 2, 8192, 8192
NCORES = 8
SHARDS = NCORES // B          # query shards per batch
NQ = N // SHARDS              # queries per core
P = 128                       # queries per tile (SBUF partitions)
NT = NQ // P                  # query tiles per core
CH = 512                      # matmul free-dim chunk (one PSUM bank fp32)
NCH = M // CH                 # chunks per tile
KA = 5                        # augmented contraction dim
F32 = mybir.dt.float32
U32 = mybir.dt.uint32
EPS = 1e-8

_CACHE: dict = {}
LAST_RESULTS = None  # BassKernelResults of the most recent run (for profiling)


def _patched_drain_and_barrier(self, tick_clock, wait_clock):
    """Tile's kernel-tail drain can accumulate >2 sem waits on one Drain
    instruction, which this walrus build rejects ("Too many sync wait
    commands"). Split the waits across a chain of single-wait drains."""
    nc = self.nc
    drain_inst = nc.sync.drain()
    wait_clock.add_sem_waits(
        drain_inst.ins, tile.ScopedClock({None: tick_clock.global_clock})
    )
    si = drain_inst.ins.sync_info
    waits = list(si.on_wait or []) if si is not None else []
    if len(waits) > 1:
        si.on_wait = waits[:1]
        for w in waits[1:]:
            d2 = nc.sync.drain()
            d2.ins.sync_info = mybir.SyncInfo(on_wait=[w], on_update=[])
    nc.all_engine_barrier()
    assert self.sems is not None
    popped = nc._tile_sem_poison_stack.pop()
    assert popped is self._sem_poison
    nc.clear_and_free_semaphores(list(self.sems.allocated().values()))
    nc.all_engine_barrier()


tile.TileContext._drain_and_barrier = _patched_drain_and_barrier


def _legalize_wait_counts(nc, max_waits=1):
    """This walrus build rejects instructions carrying more than a couple of
    sem waits ("Too many sync wait commands"). Hoist extra waits onto fresh
    same-engine EventSemaphore carriers placed immediately before the
    instruction (same engine queue => identical ordering semantics)."""
    for fn in nc.m.functions:
        for bb in fn.blocks:
            out = []
            changed = False
            for ins in bb.instructions:
                si = ins.sync_info
                waits = list(si.on_wait) if (si is not None and si.on_wait) else []
                if len(waits) > max_waits:
                    extra, keep = waits[:-max_waits], waits[-max_waits:]
                    for w in extra:
                        out.append(
                            mybir.InstEventSemaphore(
                                name=f"I-{nc.next_id()}",
                                engine=ins.engine,
                                ins=[],
                                outs=[],
                                sync_info=mybir.SyncInfo(on_wait=[w], on_update=[]),
                            )
                        )
                    si.on_wait = keep
                    changed = True
                out.append(ins)
            if changed:
                bb.instructions = out


def _build_program(repeat=1):
    nc = bass.Bass("TRN2", debug=False, target_bir_lowering=False)

    lhst = nc.dram_tensor("lhst", [KA, NQ], F32, kind="ExternalInput").ap()
    rhs = nc.dram_tensor("rhs", [KA, M], F32, kind="ExternalInput").ap()
    ftab = nc.dram_tensor("ftab", [M, 4], F32, kind="ExternalInput").ap()
    outf = nc.dram_tensor("outf", [NQ, 3], F32, kind="ExternalOutput").ap()

    with tile.TileContext(nc) as tc, ExitStack() as ctx:
        const_pool = ctx.enter_context(tc.tile_pool(name="const", bufs=1))
        negd2_pool = ctx.enter_context(tc.tile_pool(name="negd2", bufs=2))
        psum_pool = ctx.enter_context(tc.tile_pool(name="psum", bufs=8, space="PSUM"))
        small_pool = ctx.enter_context(tc.tile_pool(name="small", bufs=3))

        rhs_sb = const_pool.tile([KA, M], F32)
        nc.sync.dma_start(rhs_sb[:], rhs[:])
        lhst_sb = const_pool.tile([KA, NQ], F32)
        nc.sync.dma_start(lhst_sb[:], lhst[:])

        def tile_body(t):
            # negd2[q, j] = 2 x_q . y_j - |x_q|^2 - |y_j|^2  (= -d2)
            negd2 = negd2_pool.tile([P, M], F32)
            for c in range(NCH):
                ps = psum_pool.tile([P, CH], F32)
                nc.tensor.matmul(
                    ps[:],
                    lhst_sb[:, t * P:(t + 1) * P],
                    rhs_sb[:, c * CH:(c + 1) * CH],
                    start=True,
                    stop=True,
                )
                nc.scalar.copy(negd2[:, c * CH:(c + 1) * CH], ps[:])

            # top-8 of -d2 (descending) = 8 smallest d2; slots 0..2 are the 3-NN
            top8 = small_pool.tile([P, 8], F32)
            nc.vector.max(top8[:], negd2[:])
            idx8 = small_pool.tile([P, 8], U32)
            nc.vector.max_index(idx8[:], top8[:], negd2[:])

            # d = sqrt(max(d2, 0)); w = 1/(d + eps); w /= sum(w)
            # (smalls ride on ACT/GPSIMD so the DVE keeps its bandwidth for
            # the wide max/max_index passes; only the accuracy-critical
            # reciprocals stay on DVE)
            nd3 = small_pool.tile([P, 3], F32)
            nc.gpsimd.tensor_scalar_min(nd3[:], top8[:, 0:3], 0.0)
            d3 = small_pool.tile([P, 3], F32)
            nc.scalar.activation(
                d3[:], nd3[:], mybir.ActivationFunctionType.Sqrt, scale=-1.0
            )
            dp = small_pool.tile([P, 3], F32)
            nc.gpsimd.tensor_scalar_add(dp[:], d3[:], EPS)
            w = small_pool.tile([P, 3], F32)
            nc.vector.reciprocal(w[:], dp[:])
            wcp = small_pool.tile([P, 3], F32)
            wsum = small_pool.tile([P, 1], F32)
            nc.scalar.activation(
                wcp[:], w[:], mybir.ActivationFunctionType.Identity,
                accum_out=wsum[:],
            )
            winv = small_pool.tile([P, 1], F32)
            nc.vector.reciprocal(winv[:], wsum[:])
            wn = small_pool.tile([P, 3], F32)
            nc.scalar.mul(wn[:], w[:], winv[:, 0:1])

            # gather flow rows of the 3 nearest neighbors
            g = []
            for k in range(3):
                gk = small_pool.tile([P, 4], F32, tag=f"g{k}")
                nc.gpsimd.indirect_dma_start(
                    out=gk[:],
                    out_offset=None,
                    in_=ftab[:],
                    in_offset=bass.IndirectOffsetOnAxis(ap=idx8[:, k:k + 1], axis=0),
                )
                g.append(gk)

            # flow_out = sum_k wn_k * flow_k
            acc0 = small_pool.tile([P, 3], F32)
            nc.gpsimd.tensor_scalar_mul(acc0[:], g[0][:, 0:3], wn[:, 0:1])
            acc1 = small_pool.tile([P, 3], F32)
            nc.vector.scalar_tensor_tensor(
                acc1[:], g[1][:, 0:3], wn[:, 1:2], acc0[:],
                op0=mybir.AluOpType.mult, op1=mybir.AluOpType.add,
            )
            acc2 = small_pool.tile([P, 3], F32)
            nc.vector.scalar_tensor_tensor(
                acc2[:], g[2][:, 0:3], wn[:, 2:3], acc1[:],
                op0=mybir.AluOpType.mult, op1=mybir.AluOpType.add,
            )
            nc.sync.dma_start(outf[t * P:(t + 1) * P, :], acc2[:])

        if repeat > 1:
            with tc.For_i(0, repeat, 1):
                for t in range(NT):
                    tile_body(t)
        else:
            for t in range(NT):
                tile_body(t)

    _legalize_wait_counts(nc)
    return nc


def _get_nc():
    if "nc" not in _CACHE:
        _CACHE["nc"] = _build_program()
    return _CACHE["nc"]


def _host_prep(pc0, pc1, flow1, pose0, pose1):
    """fp32 pose math + augmented operand construction (matches reference)."""
    pc0 = np.asarray(pc0, dtype=np.float32)
    pc1 = np.asarray(pc1, dtype=np.float32)
    flow1 = np.asarray(flow1, dtype=np.float32)
    pose0 = np.asarray(pose0, dtype=np.float32)
    pose1 = np.asarray(pose1, dtype=np.float32)

    pose_0to1 = (np.linalg.inv(pose1) @ pose0).astype(np.float32)
    R = pose_0to1[:, :3, :3]
    t = pose_0to1[:, :3, 3]
    pc0_t = (np.einsum("bij,bnj->bni", R, pc0) + t[:, None, :]).astype(np.float32)
    pose_flow = pc0_t - pc0

    q2 = np.sum(pc0_t * pc0_t, axis=-1)  # [B, N]
    r2 = np.sum(pc1 * pc1, axis=-1)      # [B, M]

    in_maps = []
    for core in range(NCORES):
        b, s = divmod(core, SHARDS)
        sl = slice(s * NQ, (s + 1) * NQ)
        x = pc0_t[b, sl]                          # [NQ, 3]
        lhst = np.empty((KA, NQ), np.float32)
        lhst[0:3] = (2.0 * x).T
        lhst[3] = -q2[b, sl]
        lhst[4] = -1.0
        rhs = np.empty((KA, M), np.float32)
        rhs[0:3] = pc1[b].T
        rhs[3] = 1.0
        rhs[4] = r2[b]
        ftab = np.zeros((M, 4), np.float32)
        ftab[:, 0:3] = flow1[b]
        in_maps.append({"lhst": lhst, "rhs": rhs, "ftab": ftab})
    return in_maps, pose_flow


def kernel(pc0, pc1, flow1, pose0, pose1):
    global LAST_RESULTS
    in_maps, pose_flow = _host_prep(pc0, pc1, flow1, pose0, pose1)
    nc = _get_nc()
    res = run_bass_kernel_spmd(nc, in_maps, list(range(NCORES)))
    LAST_RESULTS = res
    flow_interp = np.empty((B, N, 3), np.float32)
    for core in range(NCORES):
        b, s = divmod(core, SHARDS)
        flow_interp[b, s * NQ:(s + 1) * NQ] = res.results[core]["outf"]
    return flow_interp, pose_flow
